# revision 34
# baseline (speedup 1.0000x reference)
"""Trainium2 Bass kernel for masked-mean action recognition head.

Computation (per sample s):
    pooled[s] = mean(x[s, :len_s, :]) over valid frames (frame 0 if len<=1)
    out[s]    = pooled[s] @ W + b

Strategy:
  - Host: balance samples across 8 cores by total valid-frame count
    (exactly 32 samples/core), pack only the valid frames contiguously
    into a per-core buffer xp [T_pad, 1600] (~49% of the data for
    uniform lengths), and build a {0,1} mask matrix S [T_pad, 32]
    marking which slot-k sample owns packed frame t.
  - Device: stream xp through the PE:
        acc[32, 1600] += S_chunk.T @ x_chunk   (chunks of 128 frames)
    then scale rows by 1/len, transpose, and multiply by W (+b) on-chip.
  - Gather per-core [32, 60] outputs and undo the permutation.

Stream precision modes (KERNEL_MODE):
  - "hilo": x is split on the host into fp16 hi + scaled fp16 lo
    (x ~= hi + lo*2^-11). Two fp16 matmul passes accumulate into the same
    PSUM, recovering full fp32 precision (~1e-7 rel) at 1 PE cycle/row.
    Total DMA bytes equal the fp32 stream (30 MB/core).
  - "mid" (default): fp16 hi + fp8e4m3 lo, both pre-scaled by 2^11 so the
    mask weights stay exact {0,1}; 2^-11 is folded into the 1/len scale.
    3 bytes/elem (~23 MB/core), ~1.5e-5 rel err.
  - "fp16": hi stream only — halves DMA bytes (~15 MB/core), ~2e-4 rel err.
  - "f32r": fp32 data, relaxed single-pass matmul (~1.4e-4 rel err).
  - "f32": exact fp32 matmul (4 cycles/row, PE-bound).

Memory-bound regime: per-core traffic ~30 MB at ~390 GB/s => ~78 us.
"""

import math
import os

import numpy as np

import concourse.mybir as mybir
import concourse.tile as tile
from concourse import bacc
from concourse.bass_utils import run_bass_kernel_spmd

P = 128          # SBUF partitions / matmul contraction tile
JC = 1600        # num_joint * dim_emb (feature dim)
NCLS = 60        # action classes
NCORES = 8
B = 256
F = 300
SAMP = B // NCORES           # 32 samples per core
GROUP = int(os.environ.get("KERNEL_GROUP", "4"))  # 128-row chunks per DMA
ALTQ = os.environ.get("KERNEL_ALTQ", "0") == "1"  # alternate HWDGE queues
COLT = os.environ.get("KERNEL_COLT", "0") == "1"  # col-tiled stage-1 PSUM
XBUFS = int(os.environ.get("KERNEL_XBUFS", "6"))  # x-tile slots
NJ = (JC + 511) // 512       # stage-1 free-dim sections (512,512,512,64)
WCH = (JC + P - 1) // P      # stage-2 K chunks over JC (13, last is 64 rows)
LO_SHIFT = 2048.0            # 2^11: keeps the lo stream out of fp16 subnormals

MODE = os.environ.get("KERNEL_MODE", "mid")

# Set from test.py to capture an NTFF profile of the run; results of the
# last run are stored in LAST_RESULT.
TRACE = os.environ.get("KERNEL_TRACE", "0") == "1"
LAST_RESULT = None

_nc_cache: dict[tuple, object] = {}


def _group_sizes(nch: int) -> list[int]:
    """DMA group sizes: big groups for bandwidth, single-chunk tail groups
    so the PE drain after the last DMA is short."""
    tail = min(3, max(0, nch - 4))
    bulk = nch - tail
    sizes = [GROUP] * (bulk // GROUP)
    if bulk % GROUP:
        sizes.append(bulk % GROUP)
    sizes += [1] * tail
    return sizes


def _build_nc(nch: int, mode: str):
    """Build the per-core Bass program for nch 128-frame chunks."""
    f32 = mybir.dt.float32
    sd = {"hilo": mybir.dt.float16, "mid": mybir.dt.float16,
          "fp16": mybir.dt.float16,
          "f32r": mybir.dt.float32r, "f32": mybir.dt.float32}[mode]
    use_lo = mode == "hilo"    # lo stream interleaved, same dtype as hi
    use_q = mode == "mid"      # lo stream in a separate fp8 tensor
    qd = mybir.dt.float8e4
    nc = bacc.Bacc("TRN2", target_bir_lowering=False, debug=False,
                   num_devices=NCORES)

    # x stream, host-rearranged to partition-major [P, nch, (streams), JC]
    # so every DMA descriptor reads a large contiguous piece per partition.
    nstr = 2 if use_lo else 1
    xp_d = nc.dram_tensor("xp", [P, nch, nstr, JC], sd, kind="ExternalInput")
    s0_d = nc.dram_tensor("s0", [P, nch, SAMP], sd, kind="ExternalInput")
    if use_q:
        xq_d = nc.dram_tensor("xq", [P, nch, JC], qd, kind="ExternalInput")
        s1_d = nc.dram_tensor("s1", [P, nch, SAMP], qd, kind="ExternalInput")
    elif use_lo:
        s1_d = nc.dram_tensor("s1", [P, nch, SAMP], sd, kind="ExternalInput")
    w_d = nc.dram_tensor("w_re", [P, WCH, NCLS], f32, kind="ExternalInput")
    b_d = nc.dram_tensor("b_rep", [SAMP, NCLS], f32, kind="ExternalInput")
    il_d = nc.dram_tensor("invlen", [P, 1], f32, kind="ExternalInput")
    id_d = nc.dram_tensor("ident", [P, SAMP], f32, kind="ExternalInput")
    o_d = nc.dram_tensor("out", [SAMP, NCLS], f32, kind="ExternalOutput")

    with tile.TileContext(nc) as tc:
        with tc.tile_pool(name="consts", bufs=1) as cpool, \
             tc.tile_pool(name="xbufs", bufs=XBUFS) as xpool, \
             tc.tile_pool(name="tail", bufs=1) as tpool, \
             tc.tile_pool(name="acc", bufs=1, space="PSUM") as apool, \
             tc.tile_pool(name="tps", bufs=2, space="PSUM") as tppool:

            # Constants ride the SWDGE (gpsimd) queue so the HWDGE queue
            # starts streaming x immediately.
            s0_sb = cpool.tile([P, nch, SAMP], sd, tag="s0_sb")
            nc.gpsimd.dma_start(out=s0_sb, in_=s0_d.ap())
            if use_lo or use_q:
                s1_sb = cpool.tile([P, nch, SAMP], qd if use_q else sd,
                                   tag="s1_sb")
                nc.gpsimd.dma_start(out=s1_sb, in_=s1_d.ap())
            w_sb = cpool.tile([P, WCH, NCLS], f32, tag="w_sb")
            nc.gpsimd.dma_start(out=w_sb, in_=w_d.ap())
            b_sb = cpool.tile([SAMP, NCLS], f32, tag="b_sb")
            nc.gpsimd.dma_start(out=b_sb, in_=b_d.ap())
            il_sb = cpool.tile([P, 1], f32, tag="il_sb")
            nc.gpsimd.dma_start(out=il_sb, in_=il_d.ap())
            # Identity (I32 tiled 4x down the partitions, for row-tiled
            # transposes in COLT mode).
            ident = cpool.tile([P, SAMP], f32, tag="ident")
            nc.gpsimd.dma_start(out=ident, in_=id_d.ap())

            # Stage-1 accumulators: pooled [32 samples, JC] in PSUM.
            # COLT: one [128, 512] bank, jj-section at partition block 32*jj,
            # written by concurrent col-tiled matmuls.
            if COLT:
                acc4 = apool.tile([P, 512], f32, tag="acc4", name="acc4")
                acc = [acc4[32 * jj:32 * jj + 32, :min(512, JC - jj * 512)]
                       for jj in range(NJ)]
            else:
                acc = []
                for jj in range(NJ):
                    nsz = min(512, JC - jj * 512)
                    acc.append(apool.tile([SAMP, nsz], f32, tag=f"acc{jj}",
                                          name=f"acc{jj}"))

            xp_ap = xp_d.ap()
            xq_ap = xq_d.ap() if use_q else None
            two_pass = use_lo or use_q
            c0 = 0
            for gi, gsz in enumerate(_group_sizes(nch)):
                xt = xpool.tile([P, GROUP, nstr, JC], sd, tag="xt")
                dma_eng = nc.scalar if (ALTQ and gi % 2) else nc.sync
                dma_eng.dma_start(
                    out=xt[:, :gsz, :, :],
                    in_=xp_ap[:, c0:c0 + gsz, :, :],
                )
                if use_q:
                    xq = xpool.tile([P, GROUP, JC], qd, tag="xq")
                    dma_eng.dma_start(
                        out=xq[:, :gsz, :],
                        in_=xq_ap[:, c0:c0 + gsz, :],
                    )
                for k in range(gsz):
                    ch = c0 + k
                    for jj in range(NJ):
                        n0 = jj * 512
                        nsz = min(512, JC - n0)
                        tp = (0, 32 * jj) if COLT else None
                        nc.tensor.matmul(
                            out=acc[jj][:, :],
                            lhsT=s0_sb[:, ch, :],
                            rhs=xt[:, k, 0, n0:n0 + nsz],
                            start=(ch == 0),
                            stop=(ch == nch - 1 and not two_pass),
                            tile_position=tp,
                        )
                        if two_pass:
                            rhs2 = (xq[:, k, n0:n0 + nsz] if use_q
                                    else xt[:, k, 1, n0:n0 + nsz])
                            nc.tensor.matmul(
                                out=acc[jj][:, :],
                                lhsT=s1_sb[:, ch, :],
                                rhs=rhs2,
                                start=False,
                                stop=(ch == nch - 1),
                                tile_position=tp,
                            )
                c0 += gsz

            # pooled = acc / len  (per-partition scalar multiply), then
            # transpose pooled -> poolT chunks [128, 32] so the PE can
            # contract over JC.
            pt_all = tpool.tile([P, WCH, SAMP], f32, tag="pt_all")
            if COLT:
                pooled4_sb = tpool.tile([P, 512], f32, tag="pooled4")
                nc.vector.tensor_scalar_mul(out=pooled4_sb[:96, :],
                                            in0=acc4[:96, :],
                                            scalar1=il_sb[:96, 0:1])
                nc.vector.tensor_scalar_mul(out=pooled4_sb[96:, :64],
                                            in0=acc4[96:, :64],
                                            scalar1=il_sb[96:, 0:1])
                # Emit transposes so consecutive ones hit distinct row
                # groups (they execute concurrently in the PE array).
                order = [c for r in range(4) for c in range(r, WCH, 4)]
                for c in order:
                    jj, col0 = c // 4, 128 * (c % 4)
                    rows = min(P, JC - c * P)
                    pt_ps = tppool.tile([P, SAMP], f32, tag="pt", bufs=4)
                    nc.tensor.transpose(
                        out=pt_ps[:rows, :],
                        in_=pooled4_sb[32 * jj:32 * jj + 32, col0:col0 + rows],
                        identity=ident[32 * jj:32 * jj + 32, :],
                        tile_position=(32 * jj, 0),
                    )
                    nc.vector.tensor_copy(out=pt_all[:rows, c, :],
                                          in_=pt_ps[:rows, :])
            else:
                pooled_sb = tpool.tile([SAMP, JC], f32, tag="pooled")
                for jj in range(NJ):
                    n0 = jj * 512
                    nsz = min(512, JC - n0)
                    nc.vector.tensor_scalar_mul(out=pooled_sb[:, n0:n0 + nsz],
                                                in0=acc[jj][:, :],
                                                scalar1=il_sb[:SAMP, 0:1])
                for c in range(WCH):
                    rows = min(P, JC - c * P)
                    pt_ps = tppool.tile([P, SAMP], f32, tag="pt")
                    nc.tensor.transpose(
                        out=pt_ps[:rows, :],
                        in_=pooled_sb[:, c * P:c * P + rows],
                        identity=ident[:SAMP, :],
                    )
                    nc.vector.tensor_copy(out=pt_all[:rows, c, :],
                                          in_=pt_ps[:rows, :])

            out_ps = tppool.tile([SAMP, NCLS], f32, tag="out_ps", bufs=1)
            for c in range(WCH):
                rows = min(P, JC - c * P)
                nc.tensor.matmul(
                    out=out_ps[:, :],
                    lhsT=pt_all[:rows, c, :],
                    rhs=w_sb[:rows, c, :],
                    start=(c == 0),
                    stop=(c == WCH - 1),
                )

            out_sb = tpool.tile([SAMP, NCLS], f32, tag="out_sb")
            nc.vector.tensor_add(out=out_sb, in0=out_ps, in1=b_sb)
            nc.sync.dma_start(out=o_d.ap(), in_=out_sb)

    nc.compile()
    return nc


def _get_nc(nch: int, mode: str):
    key = (nch, mode)
    if key not in _nc_cache:
        _nc_cache[key] = _build_nc(nch, mode)
    return _nc_cache[key]


def kernel(**inputs) -> np.ndarray:
    global LAST_RESULT
    mode = MODE
    x = np.asarray(inputs["x"], dtype=np.float32)
    lengths = np.asarray(inputs["lengths"]).astype(np.int64).reshape(-1)
    W = np.asarray(inputs["W"], dtype=np.float32)
    b = np.asarray(inputs["b"], dtype=np.float32)
    assert x.shape == (B, F, JC), x.shape

    # Effective frames per sample: the reference takes frame 0 when <=1
    # valid frames, which equals a 1-frame mean with weight 1.
    eff = np.clip(lengths, 1, F).astype(np.int64)

    # Greedy balance: exactly SAMP samples per core, equal total frames.
    order = np.argsort(-eff, kind="stable")
    loads = np.zeros(NCORES, dtype=np.int64)
    counts = np.zeros(NCORES, dtype=np.int64)
    perm = [[] for _ in range(NCORES)]
    for s in order:
        cands = [m for m in range(NCORES) if counts[m] < SAMP]
        m = min(cands, key=lambda mm: loads[mm])
        perm[m].append(int(s))
        loads[m] += int(eff[s])
        counts[m] += 1

    nch = max(1, math.ceil(int(loads.max()) / P))
    t_pad = nch * P

    xp_all = np.zeros((NCORES, t_pad, JC), dtype=np.float32)
    s_all = np.zeros((NCORES, t_pad, SAMP), dtype=np.float32)
    invlen = np.zeros((NCORES, SAMP, 1), dtype=np.float32)
    for m in range(NCORES):
        t = 0
        for k, s in enumerate(perm[m]):
            e = int(eff[s])
            xp_all[m, t:t + e] = x[s, :e]
            s_all[m, t:t + e, k] = 1.0
            invlen[m, k, 0] = 1.0 / e
            t += e
    ident4 = np.ascontiguousarray(
        np.tile(np.eye(SAMP, dtype=np.float32), (P // SAMP, 1)))

    def rearr_s(sv):
        # [M, T_pad, 32] -> [M, P, nch, 32] (partition-major for direct DMA)
        return np.ascontiguousarray(
            sv.reshape(NCORES, nch, P, SAMP).transpose(0, 2, 1, 3))

    # Rearrange the x stream(s) to [M, P, nch, (nstr,) JC] (partition-major).
    xq = s1 = None
    if mode in ("hilo", "mid", "fp16"):
        hi = xp_all.astype(np.float16)
        s0 = rearr_s(s_all).astype(np.float16)
        nstr = 2 if mode == "hilo" else 1
        xp = np.empty((NCORES, P, nch, nstr, JC), dtype=np.float16)
        if mode == "mid":
            # Both streams pre-scaled by 2^11 (exact exponent shift for hi)
            # so mask weights stay {0,1}; 1/2^11 is folded into invlen.
            hi *= np.float16(LO_SHIFT)
            xp[:, :, :, 0, :] = hi.reshape(
                NCORES, nch, P, JC).transpose(0, 2, 1, 3)
            import ml_dtypes
            lo = (xp_all * LO_SHIFT - hi.astype(np.float32)
                  ).astype(ml_dtypes.float8_e4m3)
            xq = np.ascontiguousarray(lo.reshape(
                NCORES, nch, P, JC).transpose(0, 2, 1, 3))
            s1 = rearr_s(s_all).astype(ml_dtypes.float8_e4m3)
            invlen /= LO_SHIFT
        else:
            xp[:, :, :, 0, :] = hi.reshape(
                NCORES, nch, P, JC).transpose(0, 2, 1, 3)
            if mode == "hilo":
                lo = ((xp_all - hi.astype(np.float32)) * LO_SHIFT
                      ).astype(np.float16)
                xp[:, :, :, 1, :] = lo.reshape(
                    NCORES, nch, P, JC).transpose(0, 2, 1, 3)
                s1 = (s0.astype(np.float32) / LO_SHIFT).astype(np.float16)
    else:
        s0 = rearr_s(s_all)
        xp = np.ascontiguousarray(
            xp_all.reshape(NCORES, nch, P, 1, JC).transpose(0, 2, 1, 3, 4))

    w_pad = np.zeros((WCH * P, NCLS), dtype=np.float32)
    w_pad[:JC] = W
    w_re = np.ascontiguousarray(w_pad.reshape(WCH, P, NCLS).transpose(1, 0, 2))
    b_rep = np.ascontiguousarray(
        np.broadcast_to(b.astype(np.float32).reshape(1, NCLS), (SAMP, NCLS)))

    nc = _get_nc(nch, mode)
    # invlen per-partition vector [P, 1]: samples repeat per 32-block.
    invlen4 = np.tile(invlen, (1, P // SAMP, 1))
    in_maps = []
    for m in range(NCORES):
        im = {"xp": xp[m], "s0": s0[m], "w_re": w_re, "b_rep": b_rep,
              "invlen": invlen4[m], "ident": ident4}
        if mode == "hilo":
            im["s1"] = s1[m]
        elif mode == "mid":
            im["xq"] = xq[m]
            im["s1"] = s1[m]
        in_maps.append(im)
    res = run_bass_kernel_spmd(nc, in_maps, core_ids=list(range(NCORES)),
                               trace=TRACE)
    LAST_RESULT = res

    out_full = np.zeros((B, NCLS), dtype=np.float32)
    for m in range(NCORES):
        out_full[np.asarray(perm[m], dtype=np.int64)] = res.results[m]["out"]
    return out_full


# revision 35
# speedup vs baseline: 1.0606x; 1.0606x over previous
"""Trainium2 Bass kernel for masked-mean action recognition head.

Computation (per sample s):
    pooled[s] = mean(x[s, :len_s, :]) over valid frames (frame 0 if len<=1)
    out[s]    = pooled[s] @ W + b

Strategy:
  - Host: balance samples across 8 cores by total valid-frame count
    (exactly 32 samples/core), pack only the valid frames contiguously
    into a per-core buffer xp [T_pad, 1600] (~49% of the data for
    uniform lengths), and build a {0,1} mask matrix S [T_pad, 32]
    marking which slot-k sample owns packed frame t.
  - Device: stream xp through the PE:
        acc[32, 1600] += S_chunk.T @ x_chunk   (chunks of 128 frames)
    then scale rows by 1/len, transpose, and multiply by W (+b) on-chip.
  - Gather per-core [32, 60] outputs and undo the permutation.

Stream precision modes (KERNEL_MODE):
  - "hilo": x is split on the host into fp16 hi + scaled fp16 lo
    (x ~= hi + lo*2^-11). Two fp16 matmul passes accumulate into the same
    PSUM, recovering full fp32 precision (~1e-7 rel) at 1 PE cycle/row.
    Total DMA bytes equal the fp32 stream (30 MB/core).
  - "mid" (default): fp16 hi + fp8e4m3 lo, both pre-scaled by 2^11 so the
    mask weights stay exact {0,1}; 2^-11 is folded into the 1/len scale.
    3 bytes/elem (~23 MB/core), ~1.5e-5 rel err.
  - "fp16": hi stream only — halves DMA bytes (~15 MB/core), ~2e-4 rel err.
  - "f32r": fp32 data, relaxed single-pass matmul (~1.4e-4 rel err).
  - "f32": exact fp32 matmul (4 cycles/row, PE-bound).

Memory-bound regime: per-core traffic ~30 MB at ~390 GB/s => ~78 us.
"""

import math
import os

import numpy as np

import concourse.mybir as mybir
import concourse.tile as tile
from concourse import bacc
from concourse.bass_utils import run_bass_kernel_spmd

P = 128          # SBUF partitions / matmul contraction tile
JC = 1600        # num_joint * dim_emb (feature dim)
NCLS = 60        # action classes
NCORES = 8
B = 256
F = 300
SAMP = B // NCORES           # 32 samples per core
GROUP = int(os.environ.get("KERNEL_GROUP", "4"))  # 128-row chunks per DMA
ALTQ = os.environ.get("KERNEL_ALTQ", "0") == "1"  # alternate HWDGE queues
COLT = os.environ.get("KERNEL_COLT", "1") == "1"  # col-tiled stage-1 PSUM
XBUFS = int(os.environ.get("KERNEL_XBUFS", "6"))  # x-tile slots
NJ = (JC + 511) // 512       # stage-1 free-dim sections (512,512,512,64)
WCH = (JC + P - 1) // P      # stage-2 K chunks over JC (13, last is 64 rows)
LO_SHIFT = 2048.0            # 2^11: keeps the lo stream out of fp16 subnormals

MODE = os.environ.get("KERNEL_MODE", "mid")

# Set from test.py to capture an NTFF profile of the run; results of the
# last run are stored in LAST_RESULT.
TRACE = os.environ.get("KERNEL_TRACE", "0") == "1"
LAST_RESULT = None

_nc_cache: dict[tuple, object] = {}


def _group_sizes(nch: int) -> list[int]:
    """DMA group sizes: big groups for bandwidth, single-chunk tail groups
    so the PE drain after the last DMA is short."""
    tail = min(3, max(0, nch - 4))
    bulk = nch - tail
    sizes = [GROUP] * (bulk // GROUP)
    if bulk % GROUP:
        sizes.append(bulk % GROUP)
    sizes += [1] * tail
    return sizes


def _build_nc(nch: int, mode: str):
    """Build the per-core Bass program for nch 128-frame chunks."""
    f32 = mybir.dt.float32
    sd = {"hilo": mybir.dt.float16, "mid": mybir.dt.float16,
          "fp16": mybir.dt.float16,
          "f32r": mybir.dt.float32r, "f32": mybir.dt.float32}[mode]
    use_lo = mode == "hilo"    # lo stream interleaved, same dtype as hi
    use_q = mode == "mid"      # lo stream in a separate fp8 tensor
    qd = mybir.dt.float8e4
    nc = bacc.Bacc("TRN2", target_bir_lowering=False, debug=False,
                   num_devices=NCORES)

    # x stream, host-rearranged to partition-major [P, nch, (streams), JC]
    # so every DMA descriptor reads a large contiguous piece per partition.
    nstr = 2 if use_lo else 1
    xp_d = nc.dram_tensor("xp", [P, nch, nstr, JC], sd, kind="ExternalInput")
    s0_d = nc.dram_tensor("s0", [P, nch, SAMP], sd, kind="ExternalInput")
    if use_q:
        xq_d = nc.dram_tensor("xq", [P, nch, JC], qd, kind="ExternalInput")
        s1_d = nc.dram_tensor("s1", [P, nch, SAMP], qd, kind="ExternalInput")
    elif use_lo:
        s1_d = nc.dram_tensor("s1", [P, nch, SAMP], sd, kind="ExternalInput")
    w_d = nc.dram_tensor("w_re", [P, WCH, NCLS], f32, kind="ExternalInput")
    b_d = nc.dram_tensor("b_rep", [SAMP, NCLS], f32, kind="ExternalInput")
    il_d = nc.dram_tensor("invlen", [P, 1], f32, kind="ExternalInput")
    id_d = nc.dram_tensor("ident", [P, SAMP], f32, kind="ExternalInput")
    o_d = nc.dram_tensor("out", [SAMP, NCLS], f32, kind="ExternalOutput")

    with tile.TileContext(nc) as tc:
        with tc.tile_pool(name="consts", bufs=1) as cpool, \
             tc.tile_pool(name="xbufs", bufs=XBUFS) as xpool, \
             tc.tile_pool(name="tail", bufs=1) as tpool, \
             tc.tile_pool(name="acc", bufs=1, space="PSUM") as apool, \
             tc.tile_pool(name="tps", bufs=2, space="PSUM") as tppool:

            # Constants ride the SWDGE (gpsimd) queue so the HWDGE queue
            # starts streaming x immediately.
            s0_sb = cpool.tile([P, nch, SAMP], sd, tag="s0_sb")
            nc.gpsimd.dma_start(out=s0_sb, in_=s0_d.ap())
            if use_lo or use_q:
                s1_sb = cpool.tile([P, nch, SAMP], qd if use_q else sd,
                                   tag="s1_sb")
                nc.gpsimd.dma_start(out=s1_sb, in_=s1_d.ap())
            w_sb = cpool.tile([P, WCH, NCLS], f32, tag="w_sb")
            nc.gpsimd.dma_start(out=w_sb, in_=w_d.ap())
            b_sb = cpool.tile([SAMP, NCLS], f32, tag="b_sb")
            nc.gpsimd.dma_start(out=b_sb, in_=b_d.ap())
            il_sb = cpool.tile([P, 1], f32, tag="il_sb")
            nc.gpsimd.dma_start(out=il_sb, in_=il_d.ap())
            # Identity (I32 tiled 4x down the partitions, for row-tiled
            # transposes in COLT mode).
            ident = cpool.tile([P, SAMP], f32, tag="ident")
            nc.gpsimd.dma_start(out=ident, in_=id_d.ap())

            # Stage-1 accumulators: pooled [32 samples, JC] in PSUM.
            # COLT: one [128, 512] bank, jj-section at partition block 32*jj,
            # written by concurrent col-tiled matmuls.
            if COLT:
                acc4 = apool.tile([P, 512], f32, tag="acc4", name="acc4")
                acc = [acc4[32 * jj:32 * jj + 32, :min(512, JC - jj * 512)]
                       for jj in range(NJ)]
            else:
                acc = []
                for jj in range(NJ):
                    nsz = min(512, JC - jj * 512)
                    acc.append(apool.tile([SAMP, nsz], f32, tag=f"acc{jj}",
                                          name=f"acc{jj}"))

            xp_ap = xp_d.ap()
            xq_ap = xq_d.ap() if use_q else None
            two_pass = use_lo or use_q
            c0 = 0
            for gi, gsz in enumerate(_group_sizes(nch)):
                xt = xpool.tile([P, GROUP, nstr, JC], sd, tag="xt")
                dma_eng = nc.scalar if (ALTQ and gi % 2) else nc.sync
                dma_eng.dma_start(
                    out=xt[:, :gsz, :, :],
                    in_=xp_ap[:, c0:c0 + gsz, :, :],
                )
                if use_q:
                    xq = xpool.tile([P, GROUP, JC], qd, tag="xq")
                    dma_eng.dma_start(
                        out=xq[:, :gsz, :],
                        in_=xq_ap[:, c0:c0 + gsz, :],
                    )
                for k in range(gsz):
                    ch = c0 + k
                    for jj in range(NJ):
                        n0 = jj * 512
                        nsz = min(512, JC - n0)
                        tp = (0, 32 * jj) if COLT else None
                        nc.tensor.matmul(
                            out=acc[jj][:, :],
                            lhsT=s0_sb[:, ch, :],
                            rhs=xt[:, k, 0, n0:n0 + nsz],
                            start=(ch == 0),
                            stop=(ch == nch - 1 and not two_pass),
                            tile_position=tp,
                        )
                        if two_pass:
                            rhs2 = (xq[:, k, n0:n0 + nsz] if use_q
                                    else xt[:, k, 1, n0:n0 + nsz])
                            nc.tensor.matmul(
                                out=acc[jj][:, :],
                                lhsT=s1_sb[:, ch, :],
                                rhs=rhs2,
                                start=False,
                                stop=(ch == nch - 1),
                                tile_position=tp,
                            )
                c0 += gsz

            # pooled = acc / len  (per-partition scalar multiply), then
            # transpose pooled -> poolT chunks [128, 32] so the PE can
            # contract over JC.
            pt_all = tpool.tile([P, WCH, SAMP], f32, tag="pt_all")
            if COLT:
                pooled4_sb = tpool.tile([P, 512], f32, tag="pooled4")
                nc.vector.tensor_scalar_mul(out=pooled4_sb[:96, :],
                                            in0=acc4[:96, :],
                                            scalar1=il_sb[:96, 0:1])
                nc.vector.tensor_scalar_mul(out=pooled4_sb[96:, :64],
                                            in0=acc4[96:, :64],
                                            scalar1=il_sb[96:, 0:1])
                # Emit transposes so consecutive ones hit distinct row
                # groups (they execute concurrently in the PE array).
                order = [c for r in range(4) for c in range(r, WCH, 4)]
                for c in order:
                    jj, col0 = c // 4, 128 * (c % 4)
                    rows = min(P, JC - c * P)
                    pt_ps = tppool.tile([P, SAMP], f32, tag="pt", bufs=4)
                    nc.tensor.transpose(
                        out=pt_ps[:rows, :],
                        in_=pooled4_sb[32 * jj:32 * jj + 32, col0:col0 + rows],
                        identity=ident[32 * jj:32 * jj + 32, :],
                        tile_position=(32 * jj, 0),
                    )
                    nc.vector.tensor_copy(out=pt_all[:rows, c, :],
                                          in_=pt_ps[:rows, :])
            else:
                pooled_sb = tpool.tile([SAMP, JC], f32, tag="pooled")
                for jj in range(NJ):
                    n0 = jj * 512
                    nsz = min(512, JC - n0)
                    nc.vector.tensor_scalar_mul(out=pooled_sb[:, n0:n0 + nsz],
                                                in0=acc[jj][:, :],
                                                scalar1=il_sb[:SAMP, 0:1])
                for c in range(WCH):
                    rows = min(P, JC - c * P)
                    pt_ps = tppool.tile([P, SAMP], f32, tag="pt")
                    nc.tensor.transpose(
                        out=pt_ps[:rows, :],
                        in_=pooled_sb[:, c * P:c * P + rows],
                        identity=ident[:SAMP, :],
                    )
                    nc.vector.tensor_copy(out=pt_all[:rows, c, :],
                                          in_=pt_ps[:rows, :])

            out_ps = tppool.tile([SAMP, NCLS], f32, tag="out_ps", bufs=1)
            for c in range(WCH):
                rows = min(P, JC - c * P)
                nc.tensor.matmul(
                    out=out_ps[:, :],
                    lhsT=pt_all[:rows, c, :],
                    rhs=w_sb[:rows, c, :],
                    start=(c == 0),
                    stop=(c == WCH - 1),
                )

            out_sb = tpool.tile([SAMP, NCLS], f32, tag="out_sb")
            nc.vector.tensor_add(out=out_sb, in0=out_ps, in1=b_sb)
            nc.sync.dma_start(out=o_d.ap(), in_=out_sb)

    nc.compile()
    return nc


def _get_nc(nch: int, mode: str):
    key = (nch, mode)
    if key not in _nc_cache:
        _nc_cache[key] = _build_nc(nch, mode)
    return _nc_cache[key]


def kernel(**inputs) -> np.ndarray:
    global LAST_RESULT
    mode = MODE
    x = np.asarray(inputs["x"], dtype=np.float32)
    lengths = np.asarray(inputs["lengths"]).astype(np.int64).reshape(-1)
    W = np.asarray(inputs["W"], dtype=np.float32)
    b = np.asarray(inputs["b"], dtype=np.float32)
    assert x.shape == (B, F, JC), x.shape

    # Effective frames per sample: the reference takes frame 0 when <=1
    # valid frames, which equals a 1-frame mean with weight 1.
    eff = np.clip(lengths, 1, F).astype(np.int64)

    # Greedy balance: exactly SAMP samples per core, equal total frames.
    order = np.argsort(-eff, kind="stable")
    loads = np.zeros(NCORES, dtype=np.int64)
    counts = np.zeros(NCORES, dtype=np.int64)
    perm = [[] for _ in range(NCORES)]
    for s in order:
        cands = [m for m in range(NCORES) if counts[m] < SAMP]
        m = min(cands, key=lambda mm: loads[mm])
        perm[m].append(int(s))
        loads[m] += int(eff[s])
        counts[m] += 1

    nch = max(1, math.ceil(int(loads.max()) / P))
    t_pad = nch * P

    xp_all = np.zeros((NCORES, t_pad, JC), dtype=np.float32)
    s_all = np.zeros((NCORES, t_pad, SAMP), dtype=np.float32)
    invlen = np.zeros((NCORES, SAMP, 1), dtype=np.float32)
    for m in range(NCORES):
        t = 0
        for k, s in enumerate(perm[m]):
            e = int(eff[s])
            xp_all[m, t:t + e] = x[s, :e]
            s_all[m, t:t + e, k] = 1.0
            invlen[m, k, 0] = 1.0 / e
            t += e
    ident4 = np.ascontiguousarray(
        np.tile(np.eye(SAMP, dtype=np.float32), (P // SAMP, 1)))

    def rearr_s(sv):
        # [M, T_pad, 32] -> [M, P, nch, 32] (partition-major for direct DMA)
        return np.ascontiguousarray(
            sv.reshape(NCORES, nch, P, SAMP).transpose(0, 2, 1, 3))

    # Rearrange the x stream(s) to [M, P, nch, (nstr,) JC] (partition-major).
    xq = s1 = None
    if mode in ("hilo", "mid", "fp16"):
        hi = xp_all.astype(np.float16)
        s0 = rearr_s(s_all).astype(np.float16)
        nstr = 2 if mode == "hilo" else 1
        xp = np.empty((NCORES, P, nch, nstr, JC), dtype=np.float16)
        if mode == "mid":
            # Both streams pre-scaled by 2^11 (exact exponent shift for hi)
            # so mask weights stay {0,1}; 1/2^11 is folded into invlen.
            hi *= np.float16(LO_SHIFT)
            xp[:, :, :, 0, :] = hi.reshape(
                NCORES, nch, P, JC).transpose(0, 2, 1, 3)
            import ml_dtypes
            lo = (xp_all * LO_SHIFT - hi.astype(np.float32)
                  ).astype(ml_dtypes.float8_e4m3)
            xq = np.ascontiguousarray(lo.reshape(
                NCORES, nch, P, JC).transpose(0, 2, 1, 3))
            s1 = rearr_s(s_all).astype(ml_dtypes.float8_e4m3)
            invlen /= LO_SHIFT
        else:
            xp[:, :, :, 0, :] = hi.reshape(
                NCORES, nch, P, JC).transpose(0, 2, 1, 3)
            if mode == "hilo":
                lo = ((xp_all - hi.astype(np.float32)) * LO_SHIFT
                      ).astype(np.float16)
                xp[:, :, :, 1, :] = lo.reshape(
                    NCORES, nch, P, JC).transpose(0, 2, 1, 3)
                s1 = (s0.astype(np.float32) / LO_SHIFT).astype(np.float16)
    else:
        s0 = rearr_s(s_all)
        xp = np.ascontiguousarray(
            xp_all.reshape(NCORES, nch, P, 1, JC).transpose(0, 2, 1, 3, 4))

    w_pad = np.zeros((WCH * P, NCLS), dtype=np.float32)
    w_pad[:JC] = W
    w_re = np.ascontiguousarray(w_pad.reshape(WCH, P, NCLS).transpose(1, 0, 2))
    b_rep = np.ascontiguousarray(
        np.broadcast_to(b.astype(np.float32).reshape(1, NCLS), (SAMP, NCLS)))

    nc = _get_nc(nch, mode)
    # invlen per-partition vector [P, 1]: samples repeat per 32-block.
    invlen4 = np.tile(invlen, (1, P // SAMP, 1))
    in_maps = []
    for m in range(NCORES):
        im = {"xp": xp[m], "s0": s0[m], "w_re": w_re, "b_rep": b_rep,
              "invlen": invlen4[m], "ident": ident4}
        if mode == "hilo":
            im["s1"] = s1[m]
        elif mode == "mid":
            im["xq"] = xq[m]
            im["s1"] = s1[m]
        in_maps.append(im)
    res = run_bass_kernel_spmd(nc, in_maps, core_ids=list(range(NCORES)),
                               trace=TRACE)
    LAST_RESULT = res

    out_full = np.zeros((B, NCLS), dtype=np.float32)
    for m in range(NCORES):
        out_full[np.asarray(perm[m], dtype=np.int64)] = res.results[m]["out"]
    return out_full


# revision 36
# speedup vs baseline: 1.0702x; 1.0090x over previous
"""Trainium2 Bass kernel for masked-mean action recognition head.

Computation (per sample s):
    pooled[s] = mean(x[s, :len_s, :]) over valid frames (frame 0 if len<=1)
    out[s]    = pooled[s] @ W + b

Strategy:
  - Host: balance samples across 8 cores by total valid-frame count
    (exactly 32 samples/core), pack only the valid frames contiguously
    into a per-core buffer xp [T_pad, 1600] (~49% of the data for
    uniform lengths), and build a {0,1} mask matrix S [T_pad, 32]
    marking which slot-k sample owns packed frame t.
  - Device: stream xp through the PE:
        acc[32, 1600] += S_chunk.T @ x_chunk   (chunks of 128 frames)
    then scale rows by 1/len, transpose, and multiply by W (+b) on-chip.
  - Gather per-core [32, 60] outputs and undo the permutation.

Stream precision modes (KERNEL_MODE):
  - "hilo": x is split on the host into fp16 hi + scaled fp16 lo
    (x ~= hi + lo*2^-11). Two fp16 matmul passes accumulate into the same
    PSUM, recovering full fp32 precision (~1e-7 rel) at 1 PE cycle/row.
    Total DMA bytes equal the fp32 stream (30 MB/core).
  - "mid" (default): fp16 hi + fp8e4m3 lo, both pre-scaled by 2^11 so the
    mask weights stay exact {0,1}; 2^-11 is folded into the 1/len scale.
    3 bytes/elem (~23 MB/core), ~1.5e-5 rel err.
  - "fp16": hi stream only — halves DMA bytes (~15 MB/core), ~2e-4 rel err.
  - "f32r": fp32 data, relaxed single-pass matmul (~1.4e-4 rel err).
  - "f32": exact fp32 matmul (4 cycles/row, PE-bound).

Memory-bound regime: per-core traffic ~30 MB at ~390 GB/s => ~78 us.
"""

import math
import os

import numpy as np

import concourse.mybir as mybir
import concourse.tile as tile
from concourse import bacc
from concourse.bass_utils import run_bass_kernel_spmd

P = 128          # SBUF partitions / matmul contraction tile
JC = 1600        # num_joint * dim_emb (feature dim)
NCLS = 60        # action classes
NCORES = 8
B = 256
F = 300
SAMP = B // NCORES           # 32 samples per core
GROUP = int(os.environ.get("KERNEL_GROUP", "4"))  # 128-row chunks per DMA
ALTQ = os.environ.get("KERNEL_ALTQ", "0") == "1"  # alternate HWDGE queues
COLT = os.environ.get("KERNEL_COLT", "1") == "1"  # col-tiled stage-1 PSUM
XBUFS = int(os.environ.get("KERNEL_XBUFS", "6"))  # x-tile slots
NJ = (JC + 511) // 512       # stage-1 free-dim sections (512,512,512,64)
WCH = (JC + P - 1) // P      # stage-2 K chunks over JC (13, last is 64 rows)
LO_SHIFT = 2048.0            # 2^11: keeps the lo stream out of fp16 subnormals

MODE = os.environ.get("KERNEL_MODE", "mid")

# Set from test.py to capture an NTFF profile of the run; results of the
# last run are stored in LAST_RESULT.
TRACE = os.environ.get("KERNEL_TRACE", "0") == "1"
LAST_RESULT = None

_nc_cache: dict[tuple, object] = {}


def _group_sizes(nch: int) -> list[int]:
    """DMA group sizes: big groups for bandwidth, single-chunk tail groups
    so the PE drain after the last DMA is short."""
    tail = min(3, max(0, nch - 4))
    bulk = nch - tail
    sizes = [GROUP] * (bulk // GROUP)
    if bulk % GROUP:
        sizes.append(bulk % GROUP)
    sizes += [1] * tail
    return sizes


def _build_nc(nch: int, mode: str):
    """Build the per-core Bass program for nch 128-frame chunks."""
    f32 = mybir.dt.float32
    sd = {"hilo": mybir.dt.float16, "mid": mybir.dt.float16,
          "fp16": mybir.dt.float16,
          "f32r": mybir.dt.float32r, "f32": mybir.dt.float32}[mode]
    use_lo = mode == "hilo"    # lo stream interleaved, same dtype as hi
    use_q = mode == "mid"      # lo stream in a separate fp8 tensor
    qd = mybir.dt.float8e4
    nc = bacc.Bacc("TRN2", target_bir_lowering=False, debug=False,
                   num_devices=NCORES)

    # x stream, host-rearranged to partition-major [P, nch, (streams), JC]
    # so every DMA descriptor reads a large contiguous piece per partition.
    nstr = 2 if use_lo else 1
    xp_d = nc.dram_tensor("xp", [P, nch, nstr, JC], sd, kind="ExternalInput")
    s0_d = nc.dram_tensor("s0", [P, nch, SAMP], sd, kind="ExternalInput")
    if use_q:
        xq_d = nc.dram_tensor("xq", [P, nch, JC], qd, kind="ExternalInput")
        s1_d = nc.dram_tensor("s1", [P, nch, SAMP], qd, kind="ExternalInput")
    elif use_lo:
        s1_d = nc.dram_tensor("s1", [P, nch, SAMP], sd, kind="ExternalInput")
    w_d = nc.dram_tensor("w_re", [P, WCH, NCLS], f32, kind="ExternalInput")
    b_d = nc.dram_tensor("b_rep", [SAMP, NCLS], f32, kind="ExternalInput")
    il_d = nc.dram_tensor("invlen", [P, 1], f32, kind="ExternalInput")
    id_d = nc.dram_tensor("ident", [P, SAMP], f32, kind="ExternalInput")
    o_d = nc.dram_tensor("out", [SAMP, NCLS], f32, kind="ExternalOutput")

    with tile.TileContext(nc) as tc:
        with tc.tile_pool(name="consts", bufs=1) as cpool, \
             tc.tile_pool(name="xbufs", bufs=XBUFS) as xpool, \
             tc.tile_pool(name="tail", bufs=1) as tpool, \
             tc.tile_pool(name="acc", bufs=1, space="PSUM") as apool, \
             tc.tile_pool(name="tps", bufs=2, space="PSUM") as tppool:

            # Constants ride the SWDGE (gpsimd) queue so the HWDGE queue
            # starts streaming x immediately.
            s0_sb = cpool.tile([P, nch, SAMP], sd, tag="s0_sb")
            nc.gpsimd.dma_start(out=s0_sb, in_=s0_d.ap())
            if use_lo or use_q:
                s1_sb = cpool.tile([P, nch, SAMP], qd if use_q else sd,
                                   tag="s1_sb")
                nc.gpsimd.dma_start(out=s1_sb, in_=s1_d.ap())
            w_sb = cpool.tile([P, WCH, NCLS], f32, tag="w_sb")
            nc.gpsimd.dma_start(out=w_sb, in_=w_d.ap())
            b_sb = cpool.tile([SAMP, NCLS], f32, tag="b_sb")
            nc.gpsimd.dma_start(out=b_sb, in_=b_d.ap())
            il_sb = cpool.tile([P, 1], f32, tag="il_sb")
            nc.gpsimd.dma_start(out=il_sb, in_=il_d.ap())
            # Identity (I32 tiled 4x down the partitions, for row-tiled
            # transposes in COLT mode).
            ident = cpool.tile([P, SAMP], f32, tag="ident")
            nc.gpsimd.dma_start(out=ident, in_=id_d.ap())

            # Stage-1 accumulators: pooled [32 samples, JC] in PSUM.
            # COLT: one [128, 512] bank, jj-section at partition block 32*jj,
            # written by concurrent col-tiled matmuls.
            if COLT:
                acc4 = apool.tile([P, 512], f32, tag="acc4", name="acc4")
                acc = [acc4[32 * jj:32 * jj + 32, :min(512, JC - jj * 512)]
                       for jj in range(NJ)]
            else:
                acc = []
                for jj in range(NJ):
                    nsz = min(512, JC - jj * 512)
                    acc.append(apool.tile([SAMP, nsz], f32, tag=f"acc{jj}",
                                          name=f"acc{jj}"))

            xp_ap = xp_d.ap()
            xq_ap = xq_d.ap() if use_q else None
            two_pass = use_lo or use_q
            c0 = 0
            for gi, gsz in enumerate(_group_sizes(nch)):
                xt = xpool.tile([P, GROUP, nstr, JC], sd, tag="xt")
                dma_eng = nc.scalar if (ALTQ and gi % 2) else nc.sync
                dma_eng.dma_start(
                    out=xt[:, :gsz, :, :],
                    in_=xp_ap[:, c0:c0 + gsz, :, :],
                )
                if use_q:
                    xq = xpool.tile([P, GROUP, JC], qd, tag="xq")
                    dma_eng.dma_start(
                        out=xq[:, :gsz, :],
                        in_=xq_ap[:, c0:c0 + gsz, :],
                    )
                for k in range(gsz):
                    ch = c0 + k
                    for jj in range(NJ):
                        n0 = jj * 512
                        nsz = min(512, JC - n0)
                        tp = (0, 32 * jj) if COLT else None
                        nc.tensor.matmul(
                            out=acc[jj][:, :],
                            lhsT=s0_sb[:, ch, :],
                            rhs=xt[:, k, 0, n0:n0 + nsz],
                            start=(ch == 0),
                            stop=(ch == nch - 1 and not two_pass),
                            tile_position=tp,
                        )
                        if two_pass:
                            rhs2 = (xq[:, k, n0:n0 + nsz] if use_q
                                    else xt[:, k, 1, n0:n0 + nsz])
                            nc.tensor.matmul(
                                out=acc[jj][:, :],
                                lhsT=s1_sb[:, ch, :],
                                rhs=rhs2,
                                start=False,
                                stop=(ch == nch - 1),
                                tile_position=tp,
                            )
                c0 += gsz

            # pooled = acc / len  (per-partition scalar multiply), then
            # transpose pooled -> poolT chunks [128, 32] so the PE can
            # contract over JC.
            pt_all = tpool.tile([P, WCH, SAMP], f32, tag="pt_all")
            if COLT:
                pooled4_sb = tpool.tile([P, 512], f32, tag="pooled4")
                nc.vector.tensor_scalar_mul(out=pooled4_sb[:96, :],
                                            in0=acc4[:96, :],
                                            scalar1=il_sb[:96, 0:1])
                nc.vector.tensor_scalar_mul(out=pooled4_sb[96:, :64],
                                            in0=acc4[96:, :64],
                                            scalar1=il_sb[96:, 0:1])
                # Emit transposes so consecutive ones hit distinct row
                # groups (they execute concurrently in the PE array).
                order = [c for r in range(4) for c in range(r, WCH, 4)]
                for c in order:
                    jj, col0 = c // 4, 128 * (c % 4)
                    rows = min(P, JC - c * P)
                    pt_ps = tppool.tile([P, SAMP], f32, tag="pt", bufs=4)
                    nc.tensor.transpose(
                        out=pt_ps[:rows, :],
                        in_=pooled4_sb[32 * jj:32 * jj + 32, col0:col0 + rows],
                        identity=ident[32 * jj:32 * jj + 32, :],
                        tile_position=(32 * jj, 0),
                    )
                    nc.vector.tensor_copy(out=pt_all[:rows, c, :],
                                          in_=pt_ps[:rows, :])
            else:
                pooled_sb = tpool.tile([SAMP, JC], f32, tag="pooled")
                for jj in range(NJ):
                    n0 = jj * 512
                    nsz = min(512, JC - n0)
                    nc.vector.tensor_scalar_mul(out=pooled_sb[:, n0:n0 + nsz],
                                                in0=acc[jj][:, :],
                                                scalar1=il_sb[:SAMP, 0:1])
                for c in range(WCH):
                    rows = min(P, JC - c * P)
                    pt_ps = tppool.tile([P, SAMP], f32, tag="pt")
                    nc.tensor.transpose(
                        out=pt_ps[:rows, :],
                        in_=pooled_sb[:, c * P:c * P + rows],
                        identity=ident[:SAMP, :],
                    )
                    nc.vector.tensor_copy(out=pt_all[:rows, c, :],
                                          in_=pt_ps[:rows, :])

            if COLT:
                # Col-tiled stage-2: chunk c accumulates into partition block
                # 32*(c%4) of one [128, 60] PSUM bank; the 4 blocks run
                # concurrently in the PE array and are merged by one matmul
                # with the tiled identity as a block-sum selection matrix.
                out4_ps = tppool.tile([P, NCLS], f32, tag="out4", bufs=1)
                for c in range(WCH):
                    rows = min(P, JC - c * P)
                    q = c % 4
                    nc.tensor.matmul(
                        out=out4_ps[32 * q:32 * q + 32, :],
                        lhsT=pt_all[:rows, c, :],
                        rhs=w_sb[:rows, c, :],
                        start=(c < 4),
                        stop=(c >= WCH - 4),
                        tile_position=(0, 32 * q),
                    )
                out4_sb = tpool.tile([P, NCLS], f32, tag="out4_sb")
                nc.vector.tensor_copy(out=out4_sb, in_=out4_ps)
                out_ps = tppool.tile([SAMP, NCLS], f32, tag="out_ps", bufs=1)
                nc.tensor.matmul(out=out_ps[:, :], lhsT=ident[:, :],
                                 rhs=out4_sb[:, :], start=True, stop=True)
            else:
                out_ps = tppool.tile([SAMP, NCLS], f32, tag="out_ps", bufs=1)
                for c in range(WCH):
                    rows = min(P, JC - c * P)
                    nc.tensor.matmul(
                        out=out_ps[:, :],
                        lhsT=pt_all[:rows, c, :],
                        rhs=w_sb[:rows, c, :],
                        start=(c == 0),
                        stop=(c == WCH - 1),
                    )

            out_sb = tpool.tile([SAMP, NCLS], f32, tag="out_sb")
            nc.vector.tensor_add(out=out_sb, in0=out_ps, in1=b_sb)
            nc.sync.dma_start(out=o_d.ap(), in_=out_sb)

    nc.compile()
    return nc


def _get_nc(nch: int, mode: str):
    key = (nch, mode)
    if key not in _nc_cache:
        _nc_cache[key] = _build_nc(nch, mode)
    return _nc_cache[key]


def kernel(**inputs) -> np.ndarray:
    global LAST_RESULT
    mode = MODE
    x = np.asarray(inputs["x"], dtype=np.float32)
    lengths = np.asarray(inputs["lengths"]).astype(np.int64).reshape(-1)
    W = np.asarray(inputs["W"], dtype=np.float32)
    b = np.asarray(inputs["b"], dtype=np.float32)
    assert x.shape == (B, F, JC), x.shape

    # Effective frames per sample: the reference takes frame 0 when <=1
    # valid frames, which equals a 1-frame mean with weight 1.
    eff = np.clip(lengths, 1, F).astype(np.int64)

    # Greedy balance: exactly SAMP samples per core, equal total frames.
    order = np.argsort(-eff, kind="stable")
    loads = np.zeros(NCORES, dtype=np.int64)
    counts = np.zeros(NCORES, dtype=np.int64)
    perm = [[] for _ in range(NCORES)]
    for s in order:
        cands = [m for m in range(NCORES) if counts[m] < SAMP]
        m = min(cands, key=lambda mm: loads[mm])
        perm[m].append(int(s))
        loads[m] += int(eff[s])
        counts[m] += 1

    nch = max(1, math.ceil(int(loads.max()) / P))
    t_pad = nch * P

    xp_all = np.zeros((NCORES, t_pad, JC), dtype=np.float32)
    s_all = np.zeros((NCORES, t_pad, SAMP), dtype=np.float32)
    invlen = np.zeros((NCORES, SAMP, 1), dtype=np.float32)
    for m in range(NCORES):
        t = 0
        for k, s in enumerate(perm[m]):
            e = int(eff[s])
            xp_all[m, t:t + e] = x[s, :e]
            s_all[m, t:t + e, k] = 1.0
            invlen[m, k, 0] = 1.0 / e
            t += e
    ident4 = np.ascontiguousarray(
        np.tile(np.eye(SAMP, dtype=np.float32), (P // SAMP, 1)))

    def rearr_s(sv):
        # [M, T_pad, 32] -> [M, P, nch, 32] (partition-major for direct DMA)
        return np.ascontiguousarray(
            sv.reshape(NCORES, nch, P, SAMP).transpose(0, 2, 1, 3))

    # Rearrange the x stream(s) to [M, P, nch, (nstr,) JC] (partition-major).
    xq = s1 = None
    if mode in ("hilo", "mid", "fp16"):
        hi = xp_all.astype(np.float16)
        s0 = rearr_s(s_all).astype(np.float16)
        nstr = 2 if mode == "hilo" else 1
        xp = np.empty((NCORES, P, nch, nstr, JC), dtype=np.float16)
        if mode == "mid":
            # Both streams pre-scaled by 2^11 (exact exponent shift for hi)
            # so mask weights stay {0,1}; 1/2^11 is folded into invlen.
            hi *= np.float16(LO_SHIFT)
            xp[:, :, :, 0, :] = hi.reshape(
                NCORES, nch, P, JC).transpose(0, 2, 1, 3)
            import ml_dtypes
            lo = (xp_all * LO_SHIFT - hi.astype(np.float32)
                  ).astype(ml_dtypes.float8_e4m3)
            xq = np.ascontiguousarray(lo.reshape(
                NCORES, nch, P, JC).transpose(0, 2, 1, 3))
            s1 = rearr_s(s_all).astype(ml_dtypes.float8_e4m3)
            invlen /= LO_SHIFT
        else:
            xp[:, :, :, 0, :] = hi.reshape(
                NCORES, nch, P, JC).transpose(0, 2, 1, 3)
            if mode == "hilo":
                lo = ((xp_all - hi.astype(np.float32)) * LO_SHIFT
                      ).astype(np.float16)
                xp[:, :, :, 1, :] = lo.reshape(
                    NCORES, nch, P, JC).transpose(0, 2, 1, 3)
                s1 = (s0.astype(np.float32) / LO_SHIFT).astype(np.float16)
    else:
        s0 = rearr_s(s_all)
        xp = np.ascontiguousarray(
            xp_all.reshape(NCORES, nch, P, 1, JC).transpose(0, 2, 1, 3, 4))

    w_pad = np.zeros((WCH * P, NCLS), dtype=np.float32)
    w_pad[:JC] = W
    w_re = np.ascontiguousarray(w_pad.reshape(WCH, P, NCLS).transpose(1, 0, 2))
    b_rep = np.ascontiguousarray(
        np.broadcast_to(b.astype(np.float32).reshape(1, NCLS), (SAMP, NCLS)))

    nc = _get_nc(nch, mode)
    # invlen per-partition vector [P, 1]: samples repeat per 32-block.
    invlen4 = np.tile(invlen, (1, P // SAMP, 1))
    in_maps = []
    for m in range(NCORES):
        im = {"xp": xp[m], "s0": s0[m], "w_re": w_re, "b_rep": b_rep,
              "invlen": invlen4[m], "ident": ident4}
        if mode == "hilo":
            im["s1"] = s1[m]
        elif mode == "mid":
            im["xq"] = xq[m]
            im["s1"] = s1[m]
        in_maps.append(im)
    res = run_bass_kernel_spmd(nc, in_maps, core_ids=list(range(NCORES)),
                               trace=TRACE)
    LAST_RESULT = res

    out_full = np.zeros((B, NCLS), dtype=np.float32)
    for m in range(NCORES):
        out_full[np.asarray(perm[m], dtype=np.int64)] = res.results[m]["out"]
    return out_full
